# revision 1
# baseline (speedup 1.0000x reference)
"""Trainium2 Bass kernel for nn_DotPredictor (per-edge dot products).

score[e] = <h[src[e]], h[dst[e]]>   h: [100000, 128] f32, src/dst: [640000] i64

Strategy (8 NeuronCores, SPMD):
  - Shard edges: core c handles edges [c*80000, (c+1)*80000). h replicated.
  - Per core, edges are laid out [128 partitions, 625 columns] with
    edge = p*625 + t, so index load and score store are plain contiguous DMAs.
  - Gather h rows via gpsimd indirect DMA. HW contract (probed): ONE index
    per destination partition, dest free span streamed contiguously from
    h.flat[idx[p]*128 ...]. So each gather fetches 128 rows (one per
    partition, 64 KB). G gathers land in slices of one [128, G*128] tile.
  - Multiply on VectorE (in-place), segmented free-axis reduce on VectorE.
"""

import numpy as np

N_NODES = 100000
N_EDGES = 640000
D = 128
N_CORES = 8
E_PER_CORE = N_EDGES // N_CORES  # 80000
P = 128
T = E_PER_CORE // P  # 625 columns per partition
G = 25  # tiles (columns) per DVE batch
N_GROUPS = T // G  # 25
# Deeper work-buffer ring keeps more indirect DMAs in flight on the Pool
# queue (measured ~10-15% faster than bufs=3; SBUF still has headroom).
WORK_BUFS = 6

_cached_nc = None


def _build():
    global _cached_nc
    if _cached_nc is not None:
        return _cached_nc

    from concourse import bass, mybir
    import concourse.tile as tile

    nc = bass.Bass()
    h_ext = nc.dram_tensor("h", [N_NODES, D], mybir.dt.float32, kind="ExternalInput")
    src_ext = nc.dram_tensor("src_idx", [P, T], mybir.dt.int32, kind="ExternalInput")
    dst_ext = nc.dram_tensor("dst_idx", [P, T], mybir.dt.int32, kind="ExternalInput")
    score_ext = nc.dram_tensor(
        "score", [P, T], mybir.dt.float32, kind="ExternalOutput"
    )

    with tile.TileContext(nc) as tc:
        with (
            tc.tile_pool(name="idx", bufs=1) as idx_pool,
            tc.tile_pool(name="work", bufs=WORK_BUFS) as work_pool,
            tc.tile_pool(name="acc", bufs=1) as acc_pool,
        ):
            src_t = idx_pool.tile([P, T], mybir.dt.int32, tag="src")
            dst_t = idx_pool.tile([P, T], mybir.dt.int32, tag="dst")
            nc.sync.dma_start(out=src_t[:], in_=src_ext[:])
            nc.sync.dma_start(out=dst_t[:], in_=dst_ext[:])
            score_t = acc_pool.tile([P, T], mybir.dt.float32, tag="score")
            for g in range(N_GROUPS):
                sl = slice(g * G, (g + 1) * G)
                hu = work_pool.tile([P, G * D], mybir.dt.float32, tag="hu")
                hv = work_pool.tile([P, G * D], mybir.dt.float32, tag="hv")
                for i in range(G):
                    t = g * G + i
                    nc.gpsimd.indirect_dma_start(
                        out=hu[:, i * D : (i + 1) * D],
                        out_offset=None,
                        in_=h_ext[:],
                        in_offset=bass.IndirectOffsetOnAxis(
                            ap=src_t[:, t : t + 1], axis=0
                        ),
                    )
                    nc.gpsimd.indirect_dma_start(
                        out=hv[:, i * D : (i + 1) * D],
                        out_offset=None,
                        in_=h_ext[:],
                        in_offset=bass.IndirectOffsetOnAxis(
                            ap=dst_t[:, t : t + 1], axis=0
                        ),
                    )
                nc.vector.tensor_tensor(
                    out=hu[:], in0=hu[:], in1=hv[:], op=mybir.AluOpType.mult
                )
                nc.vector.tensor_reduce(
                    out=score_t[:, sl],
                    in_=hu[:].rearrange("p (g d) -> p g d", d=D),
                    axis=mybir.AxisListType.X,
                    op=mybir.AluOpType.add,
                )
            nc.sync.dma_start(out=score_ext[:], in_=score_t[:])

    # Runs generate_event_semaphores (TRN2 allows 1 sync-wait per inst).
    nc.finalize()
    _split_dma_waits(nc, mybir)
    _cached_nc = nc
    return nc


def _split_dma_waits(nc, mybir):
    """bacc's generate_event_semaphores splits multi-wait compute insts but
    leaves InstDMACopy/InstDrain with >1 waits, which walrus codegen rejects
    ("Too many sync wait commands"). Hoist the waits onto 2-wait-capable
    InstEventSemaphores in the same engine stream (sequencers execute their
    stream in order, so a preceding wait gates the instruction)."""
    uid = 0
    for f in nc.m.functions:
        for b in f.blocks:
            new_insts = []
            for inst in b.instructions:
                si = inst.sync_info
                if (
                    type(inst).__name__ not in ("InstEventSemaphore",)
                    and si is not None
                    and si.on_wait
                    and len(si.on_wait) > 1
                ):
                    waits = list(si.on_wait)
                    si.on_wait = []
                    for i in range(0, len(waits), 2):
                        ev = mybir.InstEventSemaphore(
                            name=f"evsem-waitsplit-{uid}",
                            engine=inst.engine,
                            sync_info=mybir.SyncInfo(
                                on_wait=waits[i : i + 2], on_update=[]
                            ),
                        )
                        nc.register_instruction(ev)
                        uid += 1
                        new_insts.append(ev)
                new_insts.append(inst)
            b.instructions = new_insts


def _pack_inputs(h, src, dst):
    h = np.ascontiguousarray(np.asarray(h), dtype=np.float32)
    src32 = np.asarray(src).astype(np.int32).reshape(N_CORES, P, T)
    dst32 = np.asarray(dst).astype(np.int32).reshape(N_CORES, P, T)
    in_maps = []
    for c in range(N_CORES):
        in_maps.append(
            {
                "h": h,
                "src_idx": np.ascontiguousarray(src32[c]),
                "dst_idx": np.ascontiguousarray(dst32[c]),
            }
        )
    return in_maps


def kernel(h, src, dst):
    nc = _build()
    in_maps = _pack_inputs(h, src, dst)
    from concourse.bass_utils import run_bass_kernel_spmd

    res = run_bass_kernel_spmd(nc, in_maps, list(range(N_CORES)))
    out = np.concatenate(
        [np.asarray(res.results[c]["score"]).reshape(-1) for c in range(N_CORES)]
    )
    return out

